# revision 32
# baseline (speedup 1.0000x reference)
"""Bidirectional LSTM (B=32, T=2048, I=256, H=128/dir) for 8 Trainium2 cores.

Sharding: data-parallel over (batch, direction) - cores 0-3 run the forward
LSTM over 8 batch rows each, cores 4-7 run the backward LSTM over the
host-flipped sequences.

Per core the nonlinear recurrence is solved with block fixed-point
iteration: time is processed in blocks of L=128 steps; within a block,
K_SWEEPS sweeps each recompute all gates with one batched matmul feedback,
apply sigmoid over the whole block at once, run the c-recurrence with the
hardware tensor_tensor_scan, and recompute h.  Error contracts ~3.7x per
sweep; K=3 with the sweep-0 clamp-tanh measures rel-l2 ~7e-3 vs the fp32
reference, under the 2e-2 gate with ~3x margin.

Engine-level design (ScalarE is the bottleneck: 5C activation elements per
block-stream; measured ~360-440us/rep vs the original 450-520us in the
same load window, cost model 260us):
 - S=4 streams of BS=2 sequences (C=256 gate columns each) whose gate
   accumulators split the 8 PSUM banks.  Streams are emitted in rounds
   with a TWO-item stagger: streams {0,2} and {1,3} pair up in-phase, so
   the ScalarE always has a second ready sigmoid queued behind the current
   one (measured ~150us better than stagger 1 or 3; stagger 0/4/6 tie
   with 2).  S=2/C=512 (bigger instructions) and S=8/C=128 both measured
   worse: 2 streams starve the in-order engines, 8 halve instruction size.
 - Middle-sweep feedback as matmul PAIRS: gates += W_hh @ h_new +
   (-W_hh) @ h_old (negated weight copy in SBUF) instead of an explicit
   h_new - h_old delta op: the DVE op and a cross-engine hop leave the
   sigmoid->sigmoid critical path, and the -W*h_old half issues early,
   right after the sweep's sigmoid reads PSUM.  Column 0 of both h tiles
   holds the same carry, so the +/- pair cancels it exactly.
 - Everything fp16 except the PSUM accumulators, the c-scan internal
   state (hardware keeps scan state fp32 regardless of operand dtype) and
   the fp32 carry: x, W in fp16 (fp16 matmul is full PE rate); gates, z,
   c, tanh, h, output all fp16 (~5e-4 rel rounding, negligible against
   the 7e-3 iteration error).  Output DMA'd as fp16 (half the bytes),
   widened to fp32 on the host.
 - z fused to one DVE op: z/2 = (sigmoid(2g) - 0.5) * sigmoid(i); the
   c-scan is linear in z so it just produces c/2, undone for free by the
   tanh's input scale=2.  The g rows of W_ih/W_hh/bias are pre-scaled by
   2 on the host so one batched sigmoid covers all four gate chunks.
 - Whole x preloaded to SBUF in chunked DMAs (contiguous >=1KB runs); the
   first chunk covers only block 0 so the graded single-shot run starts
   computing almost immediately.
 - Sweep 0's feedback tanh runs as 2*clamp(c/2, +-0.5) on the DVE (fused
   with the h multiply via scalar_tensor_tensor): its error contracts
   ~rho^2 before the output, and it removes tanh work from ScalarE, the
   bottleneck engine.
 - Final h = sigmoid(o)*tanh(c) runs on the otherwise-idle GPSIMD engine
   (off the feedback path), and the output DMA is batched every 4 blocks
   (fewer, larger transfers; 4x fewer DMA-completion semaphores).
 - K=2 sweeps measure rel-l2 2.9e-2 on CPU (fails the 2e-2 gate), so K=3
   stands.  HARD_SW0 (sweep-0 entirely via DVE/GPSIMD hard clips, 1.1e-2)
   is implemented behind a flag but regresses the cost model by ~17us
   (DVE becomes a 235us wall at 80% occupancy) - left off.

_build_nc(reps=R) emits R back-to-back repetitions of the kernel (with
per-rep carry resets, so the output stays exact) - used by test.py to
amortize the per-dispatch axon overhead when timing; the graded kernel()
path uses reps=1.
"""

import numpy as np

import concourse.bass as bass
import concourse.bacc as bacc
import concourse.tile as tile
from concourse import mybir
from concourse.bass_utils import run_bass_kernel_spmd

# Problem shapes (hardcoded per contract)
B, T, I, HS = 32, 2048, 256, 256
H = 128          # per-direction hidden
G4 = 4 * H       # 512 stacked gates
NCORES = 8
U = 8            # sequences per core
S = 4            # independent streams per core (pipelining)
BS = U // S      # sequences per stream (2)
L = 128          # time-block length
NBLK = T // L
K_SWEEPS = 3
C = BS * L       # gate columns per stream-block (256)
NHALF = 1        # column groups per stream for intra-stream pipelining
STAGGER = 2      # per-stream item offset
SPLIT_SIG = False  # sigmoid in 2 chunk-group instructions
NO_OUT_DMA = False  # timing-probe knob: skip the output DMA
GP_DELTA = False    # delta-sub on gpsimd (on the feedback critical path)
GP_OUT = False      # final h multiply on DVE: the gpsimd version adds a
                    # cross-engine hop before the carry copies that feed
                    # the next block (3 A/B batches: ~5-12us slower)
GP_COPIES = True    # block-boundary carry copies on gpsimd (False: DVE,
                    # program-order after the DVE out-mult, no sem hop)
PE_DELTA = True     # middle-sweep feedback as +W*h_new / -W*h_old matmul
                    # pairs (PE has headroom) instead of an explicit
                    # h_new-h_old DVE op on the feedback critical path
OUT_F16 = True      # output DRAM tensor fp16 (host widens to fp32)
WBUFS = 2           # work-pool double/triple buffering depth
OUT_DMA_BLKS = 4    # batch the output DMA every N blocks (fewer, larger
                    # transfers; fewer DMA-completion semaphores)
HARD_SW0 = False    # sweep-0 entirely without ScalarE: hard-sigmoid /
                    # hard-tanh via DVE/GPSIMD clips.  Needs weights
                    # pre-scaled so ifo rows carry x/4 (exact sweeps use
                    # activation scale=4) and g rows carry g/2 (scale=4
                    # gives sigmoid(2g)).  CPU study: rel-l2 1.10e-2 vs
                    # 6.4e-3 for the ScalarE sweep-0 (gate is 2e-2).
GP_MAX0 = True      # HARD_SW0: the max(.,0) half of hard-sigmoid on gpsimd
GP_CLAMP = False    # sweep-0 c-clamp + h-mult on gpsimd
GP_HM1 = False      # middle-sweep h-mult on gpsimd

# gate chunk order inside the 4*H dim: (i, f, o, g); reference order is (i, f, g, o)
PERM = [0, 1, 3, 2]

F32 = mybir.dt.float32
F16 = mybir.dt.float16

_NC_CACHE = {}


def _build_nc(k_sweeps=K_SWEEPS, reps=1):
    nc = bacc.Bacc()
    xt_h = nc.dram_tensor("xt", [2, 128, U * T], F16, kind="ExternalInput")
    wih_h = nc.dram_tensor("wih", [2, 128, G4], F16, kind="ExternalInput")
    whh_h = nc.dram_tensor("whh", [128, G4], F16, kind="ExternalInput")
    bias_h = nc.dram_tensor("bias", [1, G4], F16, kind="ExternalInput")
    out_dt = F16 if OUT_F16 else F32
    out_h = nc.dram_tensor("out", [128, U * T], out_dt, kind="ExternalOutput")

    sig = mybir.ActivationFunctionType.Sigmoid
    tanh = mybir.ActivationFunctionType.Tanh
    mult = mybir.AluOpType.mult
    add = mybir.AluOpType.add
    sub = mybir.AluOpType.subtract

    with tile.TileContext(nc) as tc:
        with (
            tc.tile_pool(name="singles", bufs=1) as singles,
            tc.tile_pool(name="work", bufs=2) as work,
            tc.tile_pool(name="psum", bufs=1, space="PSUM") as psump,
        ):
            # --- weights / constants ---
            wih_sb = singles.tile([128, 2, G4], F16, tag="wih")
            nc.sync.dma_start(out=wih_sb, in_=wih_h[:, :, :].transpose([1, 0, 2]))
            whh_sb = singles.tile([128, G4], F16, tag="whh")
            nc.sync.dma_start(out=whh_sb, in_=whh_h[:, :])
            whh_ng = None
            if PE_DELTA:
                # negated W_hh for the -W*h_old half of middle-sweep
                # feedback pairs (PSUM accumulation has no subtract mode)
                whh_ng = singles.tile([128, G4], F16, tag="whhn")
                nc.vector.tensor_scalar_mul(whh_ng, whh_sb, -1.0)
            bias_sb = singles.tile([1, G4], F16, tag="bias")
            nc.sync.dma_start(out=bias_sb, in_=bias_h[:, :])
            ones_sb = singles.tile([1, C], F16, tag="ones")
            nc.vector.memset(ones_sb.bitcast(mybir.dt.uint16), 0x3C00)

            # gate accumulators: S streams x (8/S) banks = all 8 PSUM banks
            ps = [psump.tile([128, 4, C], F32, tag=f"ps{s}", name=f"ps{s}")
                  for s in range(S)]

            # Warm-up matmuls: consume every lhsT weight tile once so later
            # matmuls inherit the weight-DMA dependencies via PE program
            # order instead of carrying their own sync waits (the LDW
            # instruction has very few wait slots).
            nc.tensor.matmul(ps[0][:, 0, :], lhsT=whh_sb[:, 0:128],
                             rhs=whh_sb[:, 0:C], start=True, stop=True,
                             skip_group_check=True)
            nc.tensor.matmul(ps[0][:, 0, :], lhsT=wih_sb[:, 0, 0:128],
                             rhs=wih_sb[:, 1, 0:C], start=True, stop=True,
                             skip_group_check=True)
            nc.tensor.matmul(ps[0][:, 0, :], lhsT=bias_sb[:, 0:128],
                             rhs=ones_sb, start=True, stop=True,
                             skip_group_check=True)

            # --- x preload: [128, 2(k), BS, T] per stream, 4 t-chunks ---
            xt_r = xt_h[:, :, :].transpose([1, 0, 2]).rearrange(
                "p k (u t) -> p k u t", u=U)
            xts = [singles.tile([128, 2, BS, T], F16, tag=f"xt{s}",
                                name=f"xt{s}") for s in range(S)]
            # first chunk covers just block 0 so compute starts right away
            # in the graded reps=1 path; the rest in big chunks
            bounds = [0, L, T // 4, T // 2, 3 * T // 4, T]
            for ch in range(len(bounds) - 1):
                c0, c1 = bounds[ch], bounds[ch + 1]
                for s in range(S):
                    u0 = s * BS
                    for k in range(2):
                        nc.sync.dma_start(
                            out=xts[s][:, k, :, c0:c1],
                            in_=xt_r[:, k, u0:u0 + BS, c0:c1],
                        )

            # --- persistent per-stream state ---
            # hs ping-pong: [carry | h(0..L-1)]; col 0 only ever holds the
            # running h carry (written at block end), cols 1..L the sweep's h.
            hs = [[singles.tile([128, BS, L + 1], F16, tag=f"hs{s}{i}",
                                name=f"hs{s}{i}") for i in range(2)]
                  for s in range(S)]
            # delta ping-pong: col 0 is always zero (carry delta).
            if not PE_DELTA:
                dlt = [[singles.tile([128, BS, L], F16, tag=f"dl{s}{i}",
                                     name=f"dl{s}{i}") for i in range(2)]
                       for s in range(S)]
                for s in range(S):
                    nc.gpsimd.memset(dlt[s][0][:, :, 0], 0.0)
                    nc.gpsimd.memset(dlt[s][1][:, :, 0], 0.0)
            carry_c = [singles.tile([128, BS], F32, tag=f"cc{s}", name=f"cc{s}")
                       for s in range(S)]

            out_r = out_h[:, :].rearrange("p (u t) -> p u t", u=U)

            # ---- per-stream item emission, streams staggered by one item
            # so xg bursts and last-sweeps of different streams spread
            # across rounds instead of clustering in-phase ----
            cpb = max(1, 512 // C)
            BH = BS // NHALF          # seqs per column group
            CH = C // NHALF           # cols per column group

            def emit_xg(s, blk):
                t0 = blk * L
                for g in range(4):
                    for k in range(2):
                        nc.tensor.matmul(
                            ps[s][:, g, :],
                            lhsT=wih_sb[:, k, g * 128:(g + 1) * 128],
                            rhs=xts[s][:, k, :, t0:t0 + L],
                            start=(k == 0 and g % cpb == 0), stop=False,
                            skip_group_check=True,
                        )
                    nc.tensor.matmul(
                        ps[s][:, g, :],
                        lhsT=bias_sb[:, g * 128:(g + 1) * 128],
                        rhs=ones_sb,
                        start=False, stop=False, skip_group_check=True,
                    )

            out_bufs = {}

            sig_scale = 4.0 if HARD_SW0 else 1.0
            mn = mybir.AluOpType.min
            mx = mybir.AluOpType.max

            def emit_sweep_pre(s, blk, sw):
                last = sw == k_sweeps - 1
                if HARD_SW0 and sw == 0 and k_sweeps > 2:
                    return _emit_hard_pre(s, blk)
                ifo_t = work.tile([128, 4, C], F16, tag=f"ifo{s}", bufs=WBUFS,
                                  name=f"ifo{s}")
                if SPLIT_SIG:
                    # chunks 0 (i) and 3 (g) first (they feed z), then {f, o}
                    nc.scalar.activation(out=ifo_t[:, 0::3, :],
                                         in_=ps[s][:, 0::3, :], func=sig,
                                         scale=sig_scale)
                    nc.scalar.activation(out=ifo_t[:, 1:3, :],
                                         in_=ps[s][:, 1:3, :], func=sig,
                                         scale=sig_scale)
                else:
                    nc.scalar.activation(out=ifo_t, in_=ps[s][:, :, :],
                                         func=sig, scale=sig_scale)
                if PE_DELTA and 0 < sw < k_sweeps - 1:
                    # -W*h_old half of the feedback pair: depends only on
                    # the sigmoid's PSUM read just above and the previous
                    # sweep's h, so the PE gets a head start while the DVE
                    # leg (z/scan/h) of this sweep still runs.  Column 0 of
                    # both hs tiles holds the same carry, so the +/- pair
                    # cancels it exactly.
                    rhs_old = hs[s][(sw + 1) % 2][:, :, 0:L]
                    for g in range(4):
                        nc.tensor.matmul(
                            ps[s][:, g, :],
                            lhsT=whh_ng[:, g * 128:(g + 1) * 128],
                            rhs=rhs_old,
                            start=False, stop=False,
                            skip_group_check=True,
                        )
                z_t = work.tile([128, C], F16, tag=f"z{s}", bufs=WBUFS,
                                name=f"z{s}")
                c_t = work.tile([128, C], F16, tag=f"c{s}", bufs=WBUFS,
                                name=f"c{s}")
                for h in range(NHALF):
                    cs = slice(h * CH, (h + 1) * CH)
                    nc.vector.scalar_tensor_tensor(
                        out=z_t[:, cs], in0=ifo_t[:, 3, cs], scalar=0.5,
                        in1=ifo_t[:, 0, cs], op0=sub, op1=mult)
                    for u in range(h * BH, (h + 1) * BH):
                        nc.vector.tensor_tensor_scan(
                            out=c_t[:, u * L:(u + 1) * L],
                            data0=ifo_t[:, 1, u * L:(u + 1) * L],
                            data1=z_t[:, u * L:(u + 1) * L],
                            initial=carry_c[s][:, u:u + 1],
                            op0=mult, op1=add,
                        )
                return ifo_t, z_t, c_t

            def _emit_hard_pre(s, blk):
                # Sweep-0 without ScalarE: PSUM ifo chunks hold x/4, g chunk
                # holds g/2.  hard-sigmoid = clip(x/4 + 0.5, 0, 1); the
                # upper clip runs fused with the +0.5, the lower max(.,0)
                # on gpsimd.  hard-tanh(g)/2 = clip(g/2, +-0.5); the upper
                # min runs standalone, the lower max fuses into the
                # z-multiply.  z here is z/2 like the exact sweeps (the
                # c-scan runs in c/2 space throughout).
                q_t = work.tile([128, 3, C], F16, tag=f"q{s}", bufs=WBUFS,
                                name=f"q{s}")
                nc.vector.tensor_scalar(q_t, ps[s][:, 0:3, :], 0.5, 1.0,
                                        op0=add, op1=mn)
                eng_m = nc.gpsimd if GP_MAX0 else nc.vector
                eng_m.tensor_scalar_max(q_t, q_t, 0.0)
                g_t = work.tile([128, C], F16, tag=f"g{s}", bufs=WBUFS,
                                name=f"g{s}")
                nc.vector.tensor_scalar_min(g_t, ps[s][:, 3, :], 0.5)
                z_t = work.tile([128, C], F16, tag=f"z{s}", bufs=WBUFS,
                                name=f"z{s}")
                nc.vector.scalar_tensor_tensor(
                    out=z_t, in0=g_t, scalar=-0.5, in1=q_t[:, 0, :],
                    op0=mx, op1=mult)
                c_t = work.tile([128, C], F16, tag=f"c{s}", bufs=WBUFS,
                                name=f"c{s}")
                for u in range(BS):
                    nc.vector.tensor_tensor_scan(
                        out=c_t[:, u * L:(u + 1) * L],
                        data0=q_t[:, 1, u * L:(u + 1) * L],
                        data1=z_t[:, u * L:(u + 1) * L],
                        initial=carry_c[s][:, u:u + 1],
                        op0=mult, op1=add,
                    )
                return q_t, z_t, c_t

            def emit_sweep_post(s, blk, sw, pre):
                t0 = blk * L
                last = sw == k_sweeps - 1
                ifo_t, z_t, c_t = pre
                o_v = ifo_t[:, 2, :].rearrange("p (u t) -> p u t", u=BS)
                if sw == 0 and k_sweeps > 2:
                    # Sweep-0 feedback h tolerates a crude tanh: its error
                    # contracts ~rho^2 (~0.07) before the output, so use
                    # 2*clamp(c/2, +-0.5) on the DVE and skip the ScalarE
                    # tanh entirely (ScalarE is the bottleneck engine).
                    cl_t = work.tile([128, C], F16, tag=f"cl{s}", bufs=WBUFS,
                                     name=f"cl{s}")
                    eng_c = nc.gpsimd if GP_CLAMP else nc.vector
                    eng_c.tensor_scalar(
                        cl_t, c_t, 0.5, -0.5,
                        op0=mybir.AluOpType.min, op1=mybir.AluOpType.max)
                    cl_v = cl_t.rearrange("p (u t) -> p u t", u=BS)
                    hsN = hs[s][0]
                    eng_c.scalar_tensor_tensor(
                        out=hsN[:, :, 1:L + 1], in0=cl_v, scalar=2.0,
                        in1=o_v, op0=mult, op1=mult)
                    rhs = hs[s][0][:, :, 0:L]
                    for g in range(4):
                        nc.tensor.matmul(
                            ps[s][:, g, :],
                            lhsT=whh_sb[:, g * 128:(g + 1) * 128],
                            rhs=rhs,
                            start=False, stop=False,
                            skip_group_check=True,
                        )
                    return
                tc_t = work.tile([128, C], F16, tag=f"tc{s}", bufs=WBUFS,
                                 name=f"tc{s}")
                for h in range(NHALF):
                    cs = slice(h * CH, (h + 1) * CH)
                    nc.scalar.activation(out=tc_t[:, cs], in_=c_t[:, cs],
                                         func=tanh, scale=2.0)
                tc_v = tc_t.rearrange("p (u t) -> p u t", u=BS)
                if last:
                    ob = blk % OUT_DMA_BLKS
                    if ob == 0:
                        out_bufs[s] = work.tile(
                            [128, BS, OUT_DMA_BLKS * L], out_dt,
                            tag=f"out{s}", bufs=WBUFS, name=f"out{s}")
                    out_t = out_bufs[s]
                    osl = out_t[:, :, ob * L:(ob + 1) * L]
                    eng_out = nc.gpsimd if GP_OUT else nc.vector
                    for h in range(NHALF):
                        us = slice(h * BH, (h + 1) * BH)
                        eng_out.tensor_mul(osl[:, us], o_v[:, us],
                                           tc_v[:, us])
                    u0 = s * BS
                    if not NO_OUT_DMA and (ob == OUT_DMA_BLKS - 1
                                           or blk == NBLK - 1):
                        g0 = blk - ob
                        nc.sync.dma_start(
                            out=out_r[:, u0:u0 + BS, g0 * L:(blk + 1) * L],
                            in_=out_t[:, :, 0:(ob + 1) * L])
                    if blk < NBLK - 1:
                        eng_cp = nc.gpsimd if GP_COPIES else nc.vector
                        eng_cp.tensor_copy(out=hs[s][0][:, :, 0],
                                           in_=osl[:, :, L - 1])
                        if PE_DELTA:
                            eng_cp.tensor_copy(out=hs[s][1][:, :, 0],
                                               in_=osl[:, :, L - 1])
                        eng_cp.tensor_copy(
                            out=carry_c[s],
                            in_=c_t.rearrange(
                                "p (u t) -> p u t", u=BS)[:, :, L - 1])
                    return
                stop_all = sw == k_sweeps - 2
                eng_d = nc.gpsimd if GP_DELTA else nc.vector
                eng_h = nc.gpsimd if GP_HM1 else nc.vector
                for h in range(NHALF):
                    us = slice(h * BH, (h + 1) * BH)
                    hsN = hs[s][sw % 2]
                    eng_h.tensor_mul(hsN[:, us, 1:L + 1], o_v[:, us],
                                     tc_v[:, us])
                    if sw == 0:
                        rhs = hs[s][0][:, us, 0:L]
                    elif PE_DELTA:
                        rhs = hs[s][sw % 2][:, us, 0:L]
                    else:
                        eng_d.tensor_sub(
                            dlt[s][sw % 2][:, us, 1:L],
                            hs[s][sw % 2][:, us, 1:L],
                            hs[s][(sw + 1) % 2][:, us, 1:L])
                        rhs = dlt[s][sw % 2][:, us, 0:L]
                    for g in range(4):
                        nc.tensor.matmul(
                            ps[s][:, g, h * CH:(h + 1) * CH],
                            lhsT=whh_sb[:, g * 128:(g + 1) * 128],
                            rhs=rhs,
                            start=False,
                            stop=(stop_all and g == 3 and h == NHALF - 1),
                            skip_group_check=True,
                        )

            total_items = NBLK * (k_sweeps + 1)
            for _rep in range(reps):
              for s in range(S):
                nc.vector.memset(carry_c[s], 0.0)
                nc.gpsimd.memset(hs[s][0][:, :, 0], 0.0)
                if PE_DELTA:
                    nc.gpsimd.memset(hs[s][1][:, :, 0], 0.0)
              for t in range(total_items + (S - 1) * STAGGER):
                  pres = {}
                  for s in range(S):
                      idx = t - s * STAGGER
                      if not (0 <= idx < total_items):
                          continue
                      blk, ph = divmod(idx, k_sweeps + 1)
                      if ph == 0:
                          emit_xg(s, blk)
                      else:
                          pres[s] = (blk, ph - 1,
                                     emit_sweep_pre(s, blk, ph - 1))
                  for s, (blk, sw, pre) in pres.items():
                      emit_sweep_post(s, blk, sw, pre)

    if not nc.is_finalized():
        nc.finalize()
    return nc


def _get_nc(reps=1):
    key = f"nc{reps}"
    if key not in _NC_CACHE:
        _NC_CACHE[key] = _build_nc(reps=reps)
    return _NC_CACHE[key]


def _flip_padded(x, lengths):
    t = np.arange(x.shape[1])[None, :]
    Ln = lengths[:, None].astype(np.int64)
    idx = np.where(t < Ln, Ln - 1 - t, t)
    return np.take_along_axis(x, idx[:, :, None], axis=1)


def _pack_weights(W_ih, W_hh, b_ih, b_hh):
    # chunk order (i, f, o, g).  Without HARD_SW0 the g chunk is pre-scaled
    # by 2 (tanh(g) = 2*sigmoid(2g) - 1 inside the fused sigmoid).  With
    # HARD_SW0, ifo rows carry x/4 and g rows g/2 (exact sweeps use the
    # activation's free scale=4, so sigmoid args are x and 2g as before;
    # sweep-0's hard clips consume the quarter/half-scaled PSUM directly).
    # All factors are powers of two - exact in fp16.
    if HARD_SW0:
        s_ifo, s_g = 0.25, 0.5
    else:
        s_ifo, s_g = 1.0, 2.0
    Wi = W_ih.reshape(4, H, I)[PERM].copy()             # [4,128,256]
    Wi[0:3] *= s_ifo
    Wi[3] *= s_g
    wih = np.ascontiguousarray(
        Wi.transpose(2, 0, 1).reshape(2, 128, G4)).astype(np.float16)
    Wh = W_hh.reshape(4, H, H)[PERM].copy()             # [4,128,128]
    Wh[0:3] *= s_ifo
    Wh[3] *= s_g
    whh = np.ascontiguousarray(
        Wh.transpose(2, 0, 1).reshape(128, G4)).astype(np.float16)
    b4 = (b_ih + b_hh).reshape(4, H)[PERM].copy()
    b4[0:3] *= s_ifo
    b4[3] *= s_g
    b = b4.reshape(1, G4).astype(np.float16)
    return wih, whh, np.ascontiguousarray(b)


def _pack_x(x_shard):
    # [U, T, I] -> [2, 128, U*T] with cols (u, t) u-major
    a = x_shard.transpose(2, 0, 1).reshape(2, 128, U * T)
    return np.ascontiguousarray(a).astype(np.float16)


def _run(inputs, trace=False):
    x = np.asarray(inputs["x"], np.float32)
    lengths = np.asarray(inputs["lengths"])
    Wf_ih = np.asarray(inputs["Wf_ih"], np.float32)
    Wf_hh = np.asarray(inputs["Wf_hh"], np.float32)
    bf_ih = np.asarray(inputs["bf_ih"], np.float32)
    bf_hh = np.asarray(inputs["bf_hh"], np.float32)
    Wb_ih = np.asarray(inputs["Wb_ih"], np.float32)
    Wb_hh = np.asarray(inputs["Wb_hh"], np.float32)
    bb_ih = np.asarray(inputs["bb_ih"], np.float32)
    bb_hh = np.asarray(inputs["bb_hh"], np.float32)

    x_rev = _flip_padded(x, lengths)
    wf = _pack_weights(Wf_ih, Wf_hh, bf_ih, bf_hh)
    wb = _pack_weights(Wb_ih, Wb_hh, bb_ih, bb_hh)

    in_maps = []
    for c in range(NCORES):
        if c < 4:
            xs = x[c * U:(c + 1) * U]
            wih, whh, b = wf
        else:
            xs = x_rev[(c - 4) * U:(c - 3) * U]
            wih, whh, b = wb
        in_maps.append({
            "xt": _pack_x(xs),
            "wih": wih,
            "whh": whh,
            "bias": b,
        })

    nc = _get_nc()
    res = run_bass_kernel_spmd(nc, in_maps, core_ids=list(range(NCORES)),
                               trace=trace)
    halves = []
    for c in range(NCORES):
        o = res.results[c]["out"].reshape(128, U, T).transpose(1, 2, 0)
        halves.append(o.astype(np.float32))
    fwd = np.concatenate(halves[0:4], axis=0)   # [32, T, 128]
    bwd = np.concatenate(halves[4:8], axis=0)   # [32, T, 128]
    out = np.concatenate([fwd, bwd], axis=-1).astype(np.float32)
    return out, res.exec_time_ns


def kernel(**inputs):
    out, _ = _run(inputs, trace=False)
    return out


# revision 36
# speedup vs baseline: 1.3421x; 1.3421x over previous
"""Bidirectional LSTM (B=32, T=2048, I=256, H=128/dir) for 8 Trainium2 cores.

Sharding: data-parallel over (batch, direction) - cores 0-3 run the forward
LSTM over 8 batch rows each, cores 4-7 run the backward LSTM over the
host-flipped sequences.

Per core the nonlinear recurrence is solved with block fixed-point
iteration: time is processed in blocks of L=128 steps; within a block,
K_SWEEPS sweeps each recompute all gates with one batched matmul feedback,
apply sigmoid over the whole block at once, run the c-recurrence with the
hardware tensor_tensor_scan, and recompute h.  Error contracts ~3.7x per
sweep; K=3 with the sweep-0 clamp-tanh measures rel-l2 ~7e-3 vs the fp32
reference, under the 2e-2 gate with ~3x margin.

Engine-level design (ScalarE is the bottleneck: 5C activation elements per
block-stream; measured ~360-440us/rep vs the original 450-520us in the
same load window, cost model 260us):
 - S=4 streams of BS=2 sequences (C=256 gate columns each) whose gate
   accumulators split the 8 PSUM banks.  Streams are emitted in rounds
   with a TWO-item stagger: streams {0,2} and {1,3} pair up in-phase, so
   the ScalarE always has a second ready sigmoid queued behind the current
   one (measured ~150us better than stagger 1 or 3; stagger 0/4/6 tie
   with 2).  S=2/C=512 (bigger instructions) and S=8/C=128 both measured
   worse: 2 streams starve the in-order engines, 8 halve instruction size.
 - Middle-sweep feedback as matmul PAIRS: gates += W_hh @ h_new +
   (-W_hh) @ h_old (negated weight copy in SBUF) instead of an explicit
   h_new - h_old delta op: the DVE op and a cross-engine hop leave the
   sigmoid->sigmoid critical path, and the -W*h_old half issues early,
   right after the sweep's sigmoid reads PSUM.  Column 0 of both h tiles
   holds the same carry, so the +/- pair cancels it exactly.
 - Everything fp16 except the PSUM accumulators, the c-scan internal
   state (hardware keeps scan state fp32 regardless of operand dtype) and
   the fp32 carry: x, W in fp16 (fp16 matmul is full PE rate); gates, z,
   c, tanh, h, output all fp16 (~5e-4 rel rounding, negligible against
   the 7e-3 iteration error).  Output DMA'd as fp16 (half the bytes),
   widened to fp32 on the host.
 - z fused to one DVE op: z/2 = (sigmoid(2g) - 0.5) * sigmoid(i); the
   c-scan is linear in z so it just produces c/2, undone for free by the
   tanh's input scale=2.  The g rows of W_ih/W_hh/bias are pre-scaled by
   2 on the host so one batched sigmoid covers all four gate chunks.
 - Whole x preloaded to SBUF in chunked DMAs (contiguous >=1KB runs); the
   first chunk covers only block 0 so the graded single-shot run starts
   computing almost immediately.
 - Sweep 0's feedback tanh runs as 2*clamp(c/2, +-0.5) on the DVE (fused
   with the h multiply via scalar_tensor_tensor): its error contracts
   ~rho^2 before the output, and it removes tanh work from ScalarE, the
   bottleneck engine.
 - Final h = sigmoid(o)*tanh(c) runs on the otherwise-idle GPSIMD engine
   (off the feedback path), and the output DMA is batched every 4 blocks
   (fewer, larger transfers; 4x fewer DMA-completion semaphores).
 - K=2 sweeps measure rel-l2 2.9e-2 on CPU (fails the 2e-2 gate), so K=3
   stands.  HARD_SW0 (sweep-0 entirely via DVE/GPSIMD hard clips, 1.1e-2)
   is implemented behind a flag but regresses the cost model by ~17us
   (DVE becomes a 235us wall at 80% occupancy) - left off.

_build_nc(reps=R) emits R back-to-back repetitions of the kernel (with
per-rep carry resets, so the output stays exact) - used by test.py to
amortize the per-dispatch axon overhead when timing; the graded kernel()
path uses reps=1.
"""

import numpy as np

import concourse.bass as bass
import concourse.bacc as bacc
import concourse.tile as tile
from concourse import mybir
from concourse.bass_utils import run_bass_kernel_spmd

# Problem shapes (hardcoded per contract)
B, T, I, HS = 32, 2048, 256, 256
H = 128          # per-direction hidden
G4 = 4 * H       # 512 stacked gates
NCORES = 8
U = 8            # sequences per core
S = 4            # independent streams per core (pipelining)
BS = U // S      # sequences per stream (2)
L = 128          # time-block length
NBLK = T // L
K_SWEEPS = 3
C = BS * L       # gate columns per stream-block (256)
NHALF = 1        # column groups per stream for intra-stream pipelining
STAGGER = 2      # per-stream item offset
SPLIT_SIG = False  # sigmoid in 2 chunk-group instructions
NO_OUT_DMA = False  # timing-probe knob: skip the output DMA
GP_DELTA = False    # delta-sub on gpsimd (on the feedback critical path)
GP_OUT = False      # final h multiply on DVE: the gpsimd version adds a
                    # cross-engine hop before the carry copies that feed
                    # the next block (3 A/B batches: ~5-12us slower)
GP_COPIES = True    # block-boundary carry copies on gpsimd (False: DVE,
                    # program-order after the DVE out-mult, no sem hop)
FOLD_XG = True      # emit block b+1's xg matmuls inside block b's
                    # last-sweep post (3 items/block instead of 4; the PE
                    # fills the matmul-free last sweep, one fewer round of
                    # pipeline latency per block; HW A/B: 357.8 vs 374.4us)
PE_DELTA = True     # middle-sweep feedback as +W*h_new / -W*h_old matmul
                    # pairs (PE has headroom) instead of an explicit
                    # h_new-h_old DVE op on the feedback critical path
OUT_F16 = True      # output DRAM tensor fp16 (host widens to fp32)
WBUFS = 2           # work-pool double/triple buffering depth
OUT_DMA_BLKS = 4    # batch the output DMA every N blocks (fewer, larger
                    # transfers; fewer DMA-completion semaphores)
HARD_SW0 = False    # sweep-0 entirely without ScalarE: hard-sigmoid /
                    # hard-tanh via DVE/GPSIMD clips.  Needs weights
                    # pre-scaled so ifo rows carry x/4 (exact sweeps use
                    # activation scale=4) and g rows carry g/2 (scale=4
                    # gives sigmoid(2g)).  CPU study: rel-l2 1.10e-2 vs
                    # 6.4e-3 for the ScalarE sweep-0 (gate is 2e-2).
GP_MAX0 = True      # HARD_SW0: the max(.,0) half of hard-sigmoid on gpsimd
GP_CLAMP = False    # sweep-0 c-clamp + h-mult on gpsimd
GP_HM1 = False      # middle-sweep h-mult on gpsimd

# gate chunk order inside the 4*H dim: (i, f, o, g); reference order is (i, f, g, o)
PERM = [0, 1, 3, 2]

F32 = mybir.dt.float32
F16 = mybir.dt.float16

_NC_CACHE = {}


def _build_nc(k_sweeps=K_SWEEPS, reps=1):
    nc = bacc.Bacc()
    xt_h = nc.dram_tensor("xt", [2, 128, U * T], F16, kind="ExternalInput")
    wih_h = nc.dram_tensor("wih", [2, 128, G4], F16, kind="ExternalInput")
    whh_h = nc.dram_tensor("whh", [128, G4], F16, kind="ExternalInput")
    bias_h = nc.dram_tensor("bias", [1, G4], F16, kind="ExternalInput")
    out_dt = F16 if OUT_F16 else F32
    out_h = nc.dram_tensor("out", [128, U * T], out_dt, kind="ExternalOutput")

    sig = mybir.ActivationFunctionType.Sigmoid
    tanh = mybir.ActivationFunctionType.Tanh
    mult = mybir.AluOpType.mult
    add = mybir.AluOpType.add
    sub = mybir.AluOpType.subtract

    with tile.TileContext(nc) as tc:
        with (
            tc.tile_pool(name="singles", bufs=1) as singles,
            tc.tile_pool(name="work", bufs=2) as work,
            tc.tile_pool(name="psum", bufs=1, space="PSUM") as psump,
        ):
            # --- weights / constants ---
            wih_sb = singles.tile([128, 2, G4], F16, tag="wih")
            nc.sync.dma_start(out=wih_sb, in_=wih_h[:, :, :].transpose([1, 0, 2]))
            whh_sb = singles.tile([128, G4], F16, tag="whh")
            nc.sync.dma_start(out=whh_sb, in_=whh_h[:, :])
            whh_ng = None
            if PE_DELTA:
                # negated W_hh for the -W*h_old half of middle-sweep
                # feedback pairs (PSUM accumulation has no subtract mode)
                whh_ng = singles.tile([128, G4], F16, tag="whhn")
                nc.vector.tensor_scalar_mul(whh_ng, whh_sb, -1.0)
            bias_sb = singles.tile([1, G4], F16, tag="bias")
            nc.sync.dma_start(out=bias_sb, in_=bias_h[:, :])
            ones_sb = singles.tile([1, C], F16, tag="ones")
            nc.vector.memset(ones_sb.bitcast(mybir.dt.uint16), 0x3C00)

            # gate accumulators: S streams x (8/S) banks = all 8 PSUM banks
            ps = [psump.tile([128, 4, C], F32, tag=f"ps{s}", name=f"ps{s}")
                  for s in range(S)]

            # Warm-up matmuls: consume every lhsT weight tile once so later
            # matmuls inherit the weight-DMA dependencies via PE program
            # order instead of carrying their own sync waits (the LDW
            # instruction has very few wait slots).
            nc.tensor.matmul(ps[0][:, 0, :], lhsT=whh_sb[:, 0:128],
                             rhs=whh_sb[:, 0:C], start=True, stop=True,
                             skip_group_check=True)
            nc.tensor.matmul(ps[0][:, 0, :], lhsT=wih_sb[:, 0, 0:128],
                             rhs=wih_sb[:, 1, 0:C], start=True, stop=True,
                             skip_group_check=True)
            nc.tensor.matmul(ps[0][:, 0, :], lhsT=bias_sb[:, 0:128],
                             rhs=ones_sb, start=True, stop=True,
                             skip_group_check=True)

            # --- x preload: [128, 2(k), BS, T] per stream, 4 t-chunks ---
            xt_r = xt_h[:, :, :].transpose([1, 0, 2]).rearrange(
                "p k (u t) -> p k u t", u=U)
            xts = [singles.tile([128, 2, BS, T], F16, tag=f"xt{s}",
                                name=f"xt{s}") for s in range(S)]
            # first chunk covers just block 0 so compute starts right away
            # in the graded reps=1 path; the rest in big chunks
            bounds = [0, L, T // 4, T // 2, 3 * T // 4, T]
            for ch in range(len(bounds) - 1):
                c0, c1 = bounds[ch], bounds[ch + 1]
                for s in range(S):
                    u0 = s * BS
                    for k in range(2):
                        nc.sync.dma_start(
                            out=xts[s][:, k, :, c0:c1],
                            in_=xt_r[:, k, u0:u0 + BS, c0:c1],
                        )

            # --- persistent per-stream state ---
            # hs ping-pong: [carry | h(0..L-1)]; col 0 only ever holds the
            # running h carry (written at block end), cols 1..L the sweep's h.
            hs = [[singles.tile([128, BS, L + 1], F16, tag=f"hs{s}{i}",
                                name=f"hs{s}{i}") for i in range(2)]
                  for s in range(S)]
            # delta ping-pong: col 0 is always zero (carry delta).
            if not PE_DELTA:
                dlt = [[singles.tile([128, BS, L], F16, tag=f"dl{s}{i}",
                                     name=f"dl{s}{i}") for i in range(2)]
                       for s in range(S)]
                for s in range(S):
                    nc.gpsimd.memset(dlt[s][0][:, :, 0], 0.0)
                    nc.gpsimd.memset(dlt[s][1][:, :, 0], 0.0)
            carry_c = [singles.tile([128, BS], F32, tag=f"cc{s}", name=f"cc{s}")
                       for s in range(S)]

            out_r = out_h[:, :].rearrange("p (u t) -> p u t", u=U)

            # ---- per-stream item emission, streams staggered by one item
            # so xg bursts and last-sweeps of different streams spread
            # across rounds instead of clustering in-phase ----
            cpb = max(1, 512 // C)
            BH = BS // NHALF          # seqs per column group
            CH = C // NHALF           # cols per column group

            def emit_xg(s, blk):
                t0 = blk * L
                for g in range(4):
                    for k in range(2):
                        nc.tensor.matmul(
                            ps[s][:, g, :],
                            lhsT=wih_sb[:, k, g * 128:(g + 1) * 128],
                            rhs=xts[s][:, k, :, t0:t0 + L],
                            start=(k == 0 and g % cpb == 0), stop=False,
                            skip_group_check=True,
                        )
                    nc.tensor.matmul(
                        ps[s][:, g, :],
                        lhsT=bias_sb[:, g * 128:(g + 1) * 128],
                        rhs=ones_sb,
                        start=False, stop=False, skip_group_check=True,
                    )

            out_bufs = {}

            sig_scale = 4.0 if HARD_SW0 else 1.0
            mn = mybir.AluOpType.min
            mx = mybir.AluOpType.max

            def emit_sweep_pre(s, blk, sw):
                last = sw == k_sweeps - 1
                if HARD_SW0 and sw == 0 and k_sweeps > 2:
                    return _emit_hard_pre(s, blk)
                ifo_t = work.tile([128, 4, C], F16, tag=f"ifo{s}", bufs=WBUFS,
                                  name=f"ifo{s}")
                if SPLIT_SIG:
                    # chunks 0 (i) and 3 (g) first (they feed z), then {f, o}
                    nc.scalar.activation(out=ifo_t[:, 0::3, :],
                                         in_=ps[s][:, 0::3, :], func=sig,
                                         scale=sig_scale)
                    nc.scalar.activation(out=ifo_t[:, 1:3, :],
                                         in_=ps[s][:, 1:3, :], func=sig,
                                         scale=sig_scale)
                else:
                    nc.scalar.activation(out=ifo_t, in_=ps[s][:, :, :],
                                         func=sig, scale=sig_scale)
                if PE_DELTA and 0 < sw < k_sweeps - 1:
                    # -W*h_old half of the feedback pair: depends only on
                    # the sigmoid's PSUM read just above and the previous
                    # sweep's h, so the PE gets a head start while the DVE
                    # leg (z/scan/h) of this sweep still runs.  Column 0 of
                    # both hs tiles holds the same carry, so the +/- pair
                    # cancels it exactly.
                    rhs_old = hs[s][(sw + 1) % 2][:, :, 0:L]
                    for g in range(4):
                        nc.tensor.matmul(
                            ps[s][:, g, :],
                            lhsT=whh_ng[:, g * 128:(g + 1) * 128],
                            rhs=rhs_old,
                            start=False, stop=False,
                            skip_group_check=True,
                        )
                z_t = work.tile([128, C], F16, tag=f"z{s}", bufs=WBUFS,
                                name=f"z{s}")
                c_t = work.tile([128, C], F16, tag=f"c{s}", bufs=WBUFS,
                                name=f"c{s}")
                for h in range(NHALF):
                    cs = slice(h * CH, (h + 1) * CH)
                    nc.vector.scalar_tensor_tensor(
                        out=z_t[:, cs], in0=ifo_t[:, 3, cs], scalar=0.5,
                        in1=ifo_t[:, 0, cs], op0=sub, op1=mult)
                    for u in range(h * BH, (h + 1) * BH):
                        nc.vector.tensor_tensor_scan(
                            out=c_t[:, u * L:(u + 1) * L],
                            data0=ifo_t[:, 1, u * L:(u + 1) * L],
                            data1=z_t[:, u * L:(u + 1) * L],
                            initial=carry_c[s][:, u:u + 1],
                            op0=mult, op1=add,
                        )
                return ifo_t, z_t, c_t

            def _emit_hard_pre(s, blk):
                # Sweep-0 without ScalarE: PSUM ifo chunks hold x/4, g chunk
                # holds g/2.  hard-sigmoid = clip(x/4 + 0.5, 0, 1); the
                # upper clip runs fused with the +0.5, the lower max(.,0)
                # on gpsimd.  hard-tanh(g)/2 = clip(g/2, +-0.5); the upper
                # min runs standalone, the lower max fuses into the
                # z-multiply.  z here is z/2 like the exact sweeps (the
                # c-scan runs in c/2 space throughout).
                q_t = work.tile([128, 3, C], F16, tag=f"q{s}", bufs=WBUFS,
                                name=f"q{s}")
                nc.vector.tensor_scalar(q_t, ps[s][:, 0:3, :], 0.5, 1.0,
                                        op0=add, op1=mn)
                eng_m = nc.gpsimd if GP_MAX0 else nc.vector
                eng_m.tensor_scalar_max(q_t, q_t, 0.0)
                g_t = work.tile([128, C], F16, tag=f"g{s}", bufs=WBUFS,
                                name=f"g{s}")
                nc.vector.tensor_scalar_min(g_t, ps[s][:, 3, :], 0.5)
                z_t = work.tile([128, C], F16, tag=f"z{s}", bufs=WBUFS,
                                name=f"z{s}")
                nc.vector.scalar_tensor_tensor(
                    out=z_t, in0=g_t, scalar=-0.5, in1=q_t[:, 0, :],
                    op0=mx, op1=mult)
                c_t = work.tile([128, C], F16, tag=f"c{s}", bufs=WBUFS,
                                name=f"c{s}")
                for u in range(BS):
                    nc.vector.tensor_tensor_scan(
                        out=c_t[:, u * L:(u + 1) * L],
                        data0=q_t[:, 1, u * L:(u + 1) * L],
                        data1=z_t[:, u * L:(u + 1) * L],
                        initial=carry_c[s][:, u:u + 1],
                        op0=mult, op1=add,
                    )
                return q_t, z_t, c_t

            def emit_sweep_post(s, blk, sw, pre):
                t0 = blk * L
                last = sw == k_sweeps - 1
                ifo_t, z_t, c_t = pre
                o_v = ifo_t[:, 2, :].rearrange("p (u t) -> p u t", u=BS)
                if sw == 0 and k_sweeps > 2:
                    # Sweep-0 feedback h tolerates a crude tanh: its error
                    # contracts ~rho^2 (~0.07) before the output, so use
                    # 2*clamp(c/2, +-0.5) on the DVE and skip the ScalarE
                    # tanh entirely (ScalarE is the bottleneck engine).
                    cl_t = work.tile([128, C], F16, tag=f"cl{s}", bufs=WBUFS,
                                     name=f"cl{s}")
                    eng_c = nc.gpsimd if GP_CLAMP else nc.vector
                    eng_c.tensor_scalar(
                        cl_t, c_t, 0.5, -0.5,
                        op0=mybir.AluOpType.min, op1=mybir.AluOpType.max)
                    cl_v = cl_t.rearrange("p (u t) -> p u t", u=BS)
                    hsN = hs[s][0]
                    eng_c.scalar_tensor_tensor(
                        out=hsN[:, :, 1:L + 1], in0=cl_v, scalar=2.0,
                        in1=o_v, op0=mult, op1=mult)
                    rhs = hs[s][0][:, :, 0:L]
                    for g in range(4):
                        nc.tensor.matmul(
                            ps[s][:, g, :],
                            lhsT=whh_sb[:, g * 128:(g + 1) * 128],
                            rhs=rhs,
                            start=False, stop=False,
                            skip_group_check=True,
                        )
                    return
                tc_t = work.tile([128, C], F16, tag=f"tc{s}", bufs=WBUFS,
                                 name=f"tc{s}")
                for h in range(NHALF):
                    cs = slice(h * CH, (h + 1) * CH)
                    nc.scalar.activation(out=tc_t[:, cs], in_=c_t[:, cs],
                                         func=tanh, scale=2.0)
                tc_v = tc_t.rearrange("p (u t) -> p u t", u=BS)
                if last:
                    ob = blk % OUT_DMA_BLKS
                    if ob == 0:
                        out_bufs[s] = work.tile(
                            [128, BS, OUT_DMA_BLKS * L], out_dt,
                            tag=f"out{s}", bufs=WBUFS, name=f"out{s}")
                    out_t = out_bufs[s]
                    osl = out_t[:, :, ob * L:(ob + 1) * L]
                    eng_out = nc.gpsimd if GP_OUT else nc.vector
                    for h in range(NHALF):
                        us = slice(h * BH, (h + 1) * BH)
                        eng_out.tensor_mul(osl[:, us], o_v[:, us],
                                           tc_v[:, us])
                    u0 = s * BS
                    if not NO_OUT_DMA and (ob == OUT_DMA_BLKS - 1
                                           or blk == NBLK - 1):
                        g0 = blk - ob
                        nc.sync.dma_start(
                            out=out_r[:, u0:u0 + BS, g0 * L:(blk + 1) * L],
                            in_=out_t[:, :, 0:(ob + 1) * L])
                    if blk < NBLK - 1:
                        eng_cp = nc.gpsimd if GP_COPIES else nc.vector
                        eng_cp.tensor_copy(out=hs[s][0][:, :, 0],
                                           in_=osl[:, :, L - 1])
                        if PE_DELTA:
                            eng_cp.tensor_copy(out=hs[s][1][:, :, 0],
                                               in_=osl[:, :, L - 1])
                        eng_cp.tensor_copy(
                            out=carry_c[s],
                            in_=c_t.rearrange(
                                "p (u t) -> p u t", u=BS)[:, :, L - 1])
                        if FOLD_XG:
                            emit_xg(s, blk + 1)
                    return
                stop_all = sw == k_sweeps - 2
                eng_d = nc.gpsimd if GP_DELTA else nc.vector
                eng_h = nc.gpsimd if GP_HM1 else nc.vector
                for h in range(NHALF):
                    us = slice(h * BH, (h + 1) * BH)
                    hsN = hs[s][sw % 2]
                    eng_h.tensor_mul(hsN[:, us, 1:L + 1], o_v[:, us],
                                     tc_v[:, us])
                    if sw == 0:
                        rhs = hs[s][0][:, us, 0:L]
                    elif PE_DELTA:
                        rhs = hs[s][sw % 2][:, us, 0:L]
                    else:
                        eng_d.tensor_sub(
                            dlt[s][sw % 2][:, us, 1:L],
                            hs[s][sw % 2][:, us, 1:L],
                            hs[s][(sw + 1) % 2][:, us, 1:L])
                        rhs = dlt[s][sw % 2][:, us, 0:L]
                    for g in range(4):
                        nc.tensor.matmul(
                            ps[s][:, g, h * CH:(h + 1) * CH],
                            lhsT=whh_sb[:, g * 128:(g + 1) * 128],
                            rhs=rhs,
                            start=False,
                            stop=(stop_all and g == 3 and h == NHALF - 1),
                            skip_group_check=True,
                        )

            if FOLD_XG:
                total_items = 1 + NBLK * k_sweeps
            else:
                total_items = NBLK * (k_sweeps + 1)
            for _rep in range(reps):
              for s in range(S):
                nc.vector.memset(carry_c[s], 0.0)
                nc.gpsimd.memset(hs[s][0][:, :, 0], 0.0)
                if PE_DELTA:
                    nc.gpsimd.memset(hs[s][1][:, :, 0], 0.0)
              for t in range(total_items + (S - 1) * STAGGER):
                  pres = {}
                  for s in range(S):
                      idx = t - s * STAGGER
                      if not (0 <= idx < total_items):
                          continue
                      if FOLD_XG:
                          if idx == 0:
                              emit_xg(s, 0)
                              continue
                          blk, sw = divmod(idx - 1, k_sweeps)
                          pres[s] = (blk, sw, emit_sweep_pre(s, blk, sw))
                      else:
                          blk, ph = divmod(idx, k_sweeps + 1)
                          if ph == 0:
                              emit_xg(s, blk)
                          else:
                              pres[s] = (blk, ph - 1,
                                         emit_sweep_pre(s, blk, ph - 1))
                  for s, (blk, sw, pre) in pres.items():
                      emit_sweep_post(s, blk, sw, pre)

    if not nc.is_finalized():
        nc.finalize()
    return nc


def _get_nc(reps=1):
    key = f"nc{reps}"
    if key not in _NC_CACHE:
        _NC_CACHE[key] = _build_nc(reps=reps)
    return _NC_CACHE[key]


def _flip_padded(x, lengths):
    t = np.arange(x.shape[1])[None, :]
    Ln = lengths[:, None].astype(np.int64)
    idx = np.where(t < Ln, Ln - 1 - t, t)
    return np.take_along_axis(x, idx[:, :, None], axis=1)


def _pack_weights(W_ih, W_hh, b_ih, b_hh):
    # chunk order (i, f, o, g).  Without HARD_SW0 the g chunk is pre-scaled
    # by 2 (tanh(g) = 2*sigmoid(2g) - 1 inside the fused sigmoid).  With
    # HARD_SW0, ifo rows carry x/4 and g rows g/2 (exact sweeps use the
    # activation's free scale=4, so sigmoid args are x and 2g as before;
    # sweep-0's hard clips consume the quarter/half-scaled PSUM directly).
    # All factors are powers of two - exact in fp16.
    if HARD_SW0:
        s_ifo, s_g = 0.25, 0.5
    else:
        s_ifo, s_g = 1.0, 2.0
    Wi = W_ih.reshape(4, H, I)[PERM].copy()             # [4,128,256]
    Wi[0:3] *= s_ifo
    Wi[3] *= s_g
    wih = np.ascontiguousarray(
        Wi.transpose(2, 0, 1).reshape(2, 128, G4)).astype(np.float16)
    Wh = W_hh.reshape(4, H, H)[PERM].copy()             # [4,128,128]
    Wh[0:3] *= s_ifo
    Wh[3] *= s_g
    whh = np.ascontiguousarray(
        Wh.transpose(2, 0, 1).reshape(128, G4)).astype(np.float16)
    b4 = (b_ih + b_hh).reshape(4, H)[PERM].copy()
    b4[0:3] *= s_ifo
    b4[3] *= s_g
    b = b4.reshape(1, G4).astype(np.float16)
    return wih, whh, np.ascontiguousarray(b)


def _pack_x(x_shard):
    # [U, T, I] -> [2, 128, U*T] with cols (u, t) u-major
    a = x_shard.transpose(2, 0, 1).reshape(2, 128, U * T)
    return np.ascontiguousarray(a).astype(np.float16)


def _run(inputs, trace=False):
    x = np.asarray(inputs["x"], np.float32)
    lengths = np.asarray(inputs["lengths"])
    Wf_ih = np.asarray(inputs["Wf_ih"], np.float32)
    Wf_hh = np.asarray(inputs["Wf_hh"], np.float32)
    bf_ih = np.asarray(inputs["bf_ih"], np.float32)
    bf_hh = np.asarray(inputs["bf_hh"], np.float32)
    Wb_ih = np.asarray(inputs["Wb_ih"], np.float32)
    Wb_hh = np.asarray(inputs["Wb_hh"], np.float32)
    bb_ih = np.asarray(inputs["bb_ih"], np.float32)
    bb_hh = np.asarray(inputs["bb_hh"], np.float32)

    x_rev = _flip_padded(x, lengths)
    wf = _pack_weights(Wf_ih, Wf_hh, bf_ih, bf_hh)
    wb = _pack_weights(Wb_ih, Wb_hh, bb_ih, bb_hh)

    in_maps = []
    for c in range(NCORES):
        if c < 4:
            xs = x[c * U:(c + 1) * U]
            wih, whh, b = wf
        else:
            xs = x_rev[(c - 4) * U:(c - 3) * U]
            wih, whh, b = wb
        in_maps.append({
            "xt": _pack_x(xs),
            "wih": wih,
            "whh": whh,
            "bias": b,
        })

    nc = _get_nc()
    res = run_bass_kernel_spmd(nc, in_maps, core_ids=list(range(NCORES)),
                               trace=trace)
    halves = []
    for c in range(NCORES):
        o = res.results[c]["out"].reshape(128, U, T).transpose(1, 2, 0)
        halves.append(o.astype(np.float32))
    fwd = np.concatenate(halves[0:4], axis=0)   # [32, T, 128]
    bwd = np.concatenate(halves[4:8], axis=0)   # [32, T, 128]
    out = np.concatenate([fwd, bwd], axis=-1).astype(np.float32)
    return out, res.exec_time_ns


def kernel(**inputs):
    out, _ = _run(inputs, trace=False)
    return out
